# revision 1
# baseline (speedup 1.0000x reference)
"""Bass/Trainium2 kernel for nn_BaselineLSTM (B=2048, T=512, H=128, twin=256).

Strategy:
  - Data-parallel: batch 2048 -> 8 cores x 256; each core runs 2 interleaved
    chunks of 128 batch (pipelining hides per-step cross-engine latency).
  - State kept transposed: hT/cT = [H=128 partitions, batch free]. Gate
    matmuls are out[gate_rows, batch] = W_slice.T.T @ hT -> no per-step
    transpose anywhere.
  - Gates land in one PSUM bank per chunk-step ordered [i|f|o|g] so a single
    merged Sigmoid ACT covers i,f,o; Tanh covers g.
  - Phase P (teacher forcing): input + bias enter via a K=2 accumulating
    matmul against packed rows [y_t; 1].
  - Phase H (autoregressive): x_t = W_out h + b_out is folded into the
    recurrence:  g = (W_hh + W_ih W_out) h + (b + W_ih b_out). No feedback
    data path; bias enters via a K=1 matmul against a ones row.
  - h lives in a 4-slot SBUF ring; predictions p_t = W_out h_t are batched
    4 steps per matmul (shifted zero-padded stationary places each group in
    its own PSUM row), flushed to DRAM every 128 steps. b_out is added on
    the host.
  - The two chunks are emitted half a step out of phase (software pipeline);
    tanh(g) is issued before the i/f/o sigmoid so the c-update chain starts
    as early as possible. All matmul/elementwise data is bf16 (PSUM
    accumulation in f32); rel err vs the f32 reference is ~0.9% of absmax.
"""

import functools

import ml_dtypes
import numpy as np

import concourse.bacc as bacc
import concourse.tile as tile
from concourse import mybir
from concourse.bass_utils import run_bass_kernel_spmd

F32 = mybir.dt.float32
BF16 = mybir.dt.bfloat16
AF = mybir.ActivationFunctionType

H = 128          # hidden
NCORES = 8
BS = 256         # batch per core
BC = 128         # batch per chunk
NCHUNK = 2

# pytorch gate order (i, f, g, o) -> kernel order (i, f, o, g)
_PERM = np.concatenate([np.arange(0, 128), np.arange(128, 256),
                        np.arange(384, 512), np.arange(256, 384)])


def _build_body(tc, d, NP, NH, EPOCH):
    nc = tc.nc
    NT = NP + NH

    import contextlib
    with contextlib.ExitStack() as ctx:
        consts = ctx.enter_context(tc.tile_pool(name="consts", bufs=1))
        state = ctx.enter_context(tc.tile_pool(name="state", bufs=1))
        work = ctx.enter_context(tc.tile_pool(name="work", bufs=3))
        gpool = ctx.enter_context(tc.tile_pool(name="gates", bufs=3, space="PSUM"))
        ppool = ctx.enter_context(tc.tile_pool(name="ppsum", bufs=1, space="PSUM"))

        # ---- constants to SBUF
        whhT_p = consts.tile([H, 4 * H], BF16, tag="whhT_p")
        whhT_h = consts.tile([H, 4 * H], BF16, tag="whhT_h")
        lp = consts.tile([2, 4 * H], BF16, tag="lp")
        lh = consts.tile([1, 4 * H], BF16, tag="lh")
        woutZ = consts.tile([H, 2 * H], BF16, tag="woutZ")
        xq = consts.tile([2, NP * BS], BF16, tag="xq")
        ones = consts.tile([1, BS], BF16, tag="ones")
        nc.vector.memset(ones, 1.0)
        nc.sync.dma_start(out=whhT_p, in_=d["whhT_p"])
        nc.sync.dma_start(out=whhT_h, in_=d["whhT_h"])
        nc.sync.dma_start(out=lp, in_=d["lp"])
        nc.sync.dma_start(out=lh, in_=d["lh"])
        nc.sync.dma_start(out=woutZ, in_=d["woutZ"])
        nc.sync.dma_start(out=xq, in_=d["xq"])

        # ---- state: h kept in a 4-slot ring (slot s%4) so predictions can
        # be batched 4 steps per matmul against consecutive slots.
        hist = []
        cT = []
        for ch in range(NCHUNK):
            hh = state.tile([H, 4 * BC], BF16, tag=f"hist{ch}")
            c = state.tile([H, BC], BF16, tag=f"cT{ch}")
            nc.vector.memset(hh, 0.0)
            nc.vector.memset(c, 0.0)
            hist.append(hh)
            cT.append(c)

        pps = [None, None]
        sigs = [None, None]
        tgs = [None, None]
        gates_l = [None, None]

        def front(s, ch):
            """Gate matmuls + sigmoid/tanh activations for step s."""
            phase_p = s < NP
            gates = gpool.tile([H, 4 * H], F32, tag=f"g{ch}",
                               name=f"g{ch}_{s}")
            gates_l[ch] = gates
            whh = whhT_p if phase_p else whhT_h
            hprev = hist[ch][:, ((s - 1) % 4) * BC: ((s - 1) % 4 + 1) * BC]

            def gate_mm(k):
                go = gates[:, k * H:(k + 1) * H]
                nc.tensor.matmul(go, whh[:, k * H:(k + 1) * H], hprev,
                                 start=True, stop=False)
                if phase_p:
                    rhs = xq[0:2, s * BS + ch * BC: s * BS + ch * BC + BC]
                    lhs2 = lp[0:2, k * H:(k + 1) * H]
                else:
                    rhs = ones[0:1, ch * BC: ch * BC + BC]
                    lhs2 = lh[0:1, k * H:(k + 1) * H]
                nc.tensor.matmul(go, lhs2, rhs, start=False, stop=True)

            # g-gate first so tanh(g) can run on ACT while i/f/o matmuls
            # are still streaming; sigmoid follows.
            gate_mm(3)
            tg = work.tile([H, BC], BF16, tag=f"tg{ch}", name=f"tg{ch}_{s}")
            nc.scalar.activation(tg, gates[:, 3 * H:4 * H], AF.Tanh)
            for k in (0, 1, 2):
                gate_mm(k)
            sig = work.tile([H, 3 * H], BF16, tag=f"sig{ch}",
                            name=f"sig{ch}_{s}")
            nc.scalar.activation(sig, gates[:, 0:3 * H], AF.Sigmoid)
            sigs[ch] = sig
            tgs[ch] = tg

        def back(s, ch):
            """c/h update for step s + batched prediction matmul."""
            sig, tg = sigs[ch], tgs[ch]
            t2 = work.tile([H, BC], BF16, tag=f"t2{ch}", name=f"t2{ch}_{s}")
            nc.vector.tensor_mul(t2, sig[:, H:2 * H], cT[ch])
            t1 = work.tile([H, BC], BF16, tag=f"t1{ch}", name=f"t1{ch}_{s}")
            nc.gpsimd.tensor_mul(t1, sig[:, 0:H], tg)
            nc.vector.tensor_add(cT[ch], t2, t1)
            tcn = work.tile([H, BC], BF16, tag=f"tcn{ch}", name=f"tcn{ch}_{s}")
            nc.scalar.activation(tcn, cT[ch], AF.Tanh)
            hslot = hist[ch][:, (s % 4) * BC: (s % 4 + 1) * BC]
            nc.vector.tensor_mul(hslot, sig[:, 2 * H:3 * H], tcn)

            # Predictions: every 4 steps, p for steps 4G..4G+3 = one matmul
            # W_out @ [h_0|h_1|h_2|h_3]; row placement via shifted zero-pad.
            if s % 4 == 3 or s == NT - 1:
                G = s // 4
                r = G % 32
                n = (s % 4 + 1) * BC
                if r == 0:
                    pps[ch] = ppool.tile([H, 4 * BC], F32, tag=f"pps{ch}",
                                         name=f"pps{ch}_{s}")
                nc.tensor.matmul(pps[ch][:, 0:n],
                                 woutZ[:, H - r: 2 * H - r],
                                 hist[ch][:, 0:n],
                                 start=(r == 0), stop=(r == 31 or s == NT - 1),
                                 skip_group_check=True)
                if r == 31 or s == NT - 1:
                    e = G // 32
                    pc = work.tile([32, 4 * BC], F32, tag=f"pc{ch}",
                                   name=f"pc{ch}_{s}")
                    nc.vector.tensor_copy(pc, pps[ch][0:32, :])
                    nc.sync.dma_start(out=d["preds"][e, ch], in_=pc)

        # Software pipeline: chunk 1 runs half a step behind chunk 0 so
        # engines ping-pong between the two independent recurrences.
        for s in range(NT):
            front(s, 0)
            if s > 0:
                back(s - 1, 1)
            front(s, 1)
            back(s, 0)
        back(NT - 1, 1)


@functools.lru_cache(maxsize=2)
def _program(NP, NH, EPOCH):
    nc = bacc.Bacc("TRN2", target_bir_lowering=False, debug=False,
                   num_devices=NCORES)
    NT = NP + NH
    NEP = (NT + 127) // 128
    d = {
        "whhT_p": nc.dram_tensor("whhT_p", [H, 4 * H], BF16,
                                 kind="ExternalInput").ap(),
        "whhT_h": nc.dram_tensor("whhT_h", [H, 4 * H], BF16,
                                 kind="ExternalInput").ap(),
        "lp": nc.dram_tensor("lp", [2, 4 * H], BF16, kind="ExternalInput").ap(),
        "lh": nc.dram_tensor("lh", [1, 4 * H], BF16, kind="ExternalInput").ap(),
        "woutZ": nc.dram_tensor("woutZ", [H, 2 * H], BF16,
                                kind="ExternalInput").ap(),
        "xq": nc.dram_tensor("xq", [2, NP * BS], BF16,
                             kind="ExternalInput").ap(),
        "preds": nc.dram_tensor("preds", [NEP, NCHUNK, 32, 4 * BC], F32,
                                kind="ExternalOutput").ap(),
    }
    with tile.TileContext(nc) as tc:
        _build_body(tc, d, NP, NH, EPOCH)
    nc.compile()
    return nc


def _host_prep(y_flow, W_ih, W_hh, b_ih, b_hh, W_out, b_out, NP):
    """Build per-core input maps. y_flow: (B, T, 1) f32."""
    bf = ml_dtypes.bfloat16
    W_ih = np.asarray(W_ih, np.float32)
    W_hh = np.asarray(W_hh, np.float32)
    W_out = np.asarray(W_out, np.float32)
    bias = np.asarray(b_ih, np.float32) + np.asarray(b_hh, np.float32)
    b_out = np.asarray(b_out, np.float32)

    W_hh_H = W_hh + W_ih @ W_out          # [4H, H]
    bias_H = bias + W_ih[:, 0] * b_out[0]

    whhT_p = np.ascontiguousarray(W_hh[_PERM].T).astype(bf)      # [H, 4H]
    whhT_h = np.ascontiguousarray(W_hh_H[_PERM].T).astype(bf)
    lp = np.stack([W_ih[_PERM, 0], bias[_PERM]]).astype(bf)       # [2, 4H]
    lh = bias_H[_PERM][None, :].astype(bf)                        # [1, 4H]
    woutZ = np.zeros((H, 2 * H), np.float32)                      # [H, 256]
    woutZ[:, H] = W_out[0]
    woutZ = woutZ.astype(bf)

    y = np.asarray(y_flow, np.float32)[:, :, 0]                   # [B, T]
    B = y.shape[0]
    in_maps = []
    for core in range(NCORES):
        yc = y[core * BS:(core + 1) * BS]                         # [BS, T]
        xq = np.ones((2, NP * BS), np.float32)
        xq[0] = yc[:, :NP].T.reshape(-1)
        in_maps.append({
            "whhT_p": whhT_p, "whhT_h": whhT_h, "lp": lp, "lh": lh,
            "woutZ": woutZ, "xq": xq.astype(bf),
        })
    return in_maps


def kernel(y_flow, x_dyn, W_ih, W_hh, b_ih, b_hh, W_out, b_out, twin_idx,
           _trace=False):
    twin = int(twin_idx)
    assert twin == 256, f"kernel hardcodes twin_idx=256, got {twin}"
    B, T, _ = y_flow.shape
    assert (B, T) == (2048, 512)
    NP, NH, EPOCH = twin - 1, T - twin, 128
    NT = NP + NH

    nc = _program(NP, NH, EPOCH)
    in_maps = _host_prep(y_flow, W_ih, W_hh, b_ih, b_hh, W_out, b_out, NP)
    res = run_bass_kernel_spmd(nc, in_maps, core_ids=list(range(NCORES)),
                               trace=_trace)

    b_out = np.asarray(b_out, np.float32)
    out = np.empty((B, NT, 1), np.float32)
    for core in range(NCORES):
        p = np.asarray(res.results[core]["preds"], np.float32)
        nep = p.shape[0]
        a = p.reshape(nep, NCHUNK, 32, 4, BC)      # [e, ch, r, j, b]
        for ch in range(NCHUNK):
            blk = a[:, ch].transpose(3, 0, 1, 2).reshape(BC, -1)[:, :NT]
            out[core * BS + ch * BC: core * BS + (ch + 1) * BC, :, 0] = \
                blk + b_out[0]
    if _trace:
        kernel._last_results = res
    return out



# revision 2
# speedup vs baseline: 1.1769x; 1.1769x over previous
"""Bass/Trainium2 kernel v2 for nn_BaselineLSTM (B=2048, T=512, H=128, twin=256).

Strategy (changes vs v1 baseline):
  - All-tanh gates: host pre-scales i/f/o rows by 0.5 so sigmoid(x) =
    (tanh(x/2)+1)/2 comes out of the SAME tanh ACT instruction as the g
    gate: ONE activation per chunk-step covers all 4 gates [128, 512].
  - Doubled state: h~ = 2h, c~ = 2c. The 0.5 factors are absorbed into
    W_hh columns / W_out on the host. Cell update is 4 element ops:
      u  = (Si + 1) * Tg            (gpsimd scalar_tensor_tensor) = 2*sig(i)*tanh(g)
      v  = (Sf*0.5 + 0.5) * c~      (DVE custom AFFINE_MUL_REDUCE) = sig(f)*c~
      c~ = u + v                    (DVE tensor_add)
      Tc = tanh(0.5 * c~)           (ACT, scale folds the 0.5)
      h~ = (So + 1) * Tc            (DVE scalar_tensor_tensor) = 2h
  - Input+bias enter via ONE K=8 matmul (N=512) per chunk-step against a
    block-pattern rhs: rows 2g hold y_t (phase P), rows 2g+1 hold ones;
    lhsT packs [W_ih_g; bias_g] per gate. Phase H uses a constant rhs
    (bias only; the p_{t-1} feedback is folded into W_hh_H).
  - Predictions: shifted zero-padded W_out stationary accumulates 4 steps
    per matmul, 32 groups per PSUM bank, flushed every 128 steps.
"""

import functools

import ml_dtypes
import numpy as np

import concourse.bacc as bacc
import concourse.tile as tile
from concourse import mybir
from concourse.bass_utils import run_bass_kernel_spmd

F32 = mybir.dt.float32
BF16 = mybir.dt.bfloat16
AF = mybir.ActivationFunctionType
ALU = mybir.AluOpType

H = 128          # hidden
NCORES = 8
BS = 256         # batch per core
BC = 128         # batch per chunk
NCHUNK = 2
BLK = 16         # xq DMA block, steps


def _build_body(tc, d, NP, NH):
    nc = tc.nc
    NT = NP + NH
    NBLK = (NP + BLK - 1) // BLK

    import contextlib
    with contextlib.ExitStack() as ctx:
        consts = ctx.enter_context(tc.tile_pool(name="consts", bufs=1))
        state = ctx.enter_context(tc.tile_pool(name="state", bufs=1))
        work = ctx.enter_context(tc.tile_pool(name="work", bufs=3))
        xpool = ctx.enter_context(tc.tile_pool(name="xq", bufs=2))
        gpool = ctx.enter_context(tc.tile_pool(name="gates", bufs=2, space="PSUM"))
        ppool = ctx.enter_context(tc.tile_pool(name="ppsum", bufs=1, space="PSUM"))

        # ---- constants to SBUF
        whhT_p = consts.tile([H, 4 * H], BF16, tag="whhT_p")
        whhT_h = consts.tile([H, 4 * H], BF16, tag="whhT_h")
        lp2_p = consts.tile([8, H], BF16, tag="lp2_p")
        lp2_h = consts.tile([8, H], BF16, tag="lp2_h")
        hq = consts.tile([8, 4 * BC], BF16, tag="hq")
        woutZ = consts.tile([H, 2 * H], BF16, tag="woutZ")
        nc.sync.dma_start(out=whhT_p, in_=d["whhT_p"])
        nc.sync.dma_start(out=whhT_h, in_=d["whhT_h"])
        nc.sync.dma_start(out=lp2_p, in_=d["lp2_p"])
        nc.sync.dma_start(out=lp2_h, in_=d["lp2_h"])
        nc.sync.dma_start(out=hq, in_=d["hq"])
        nc.sync.dma_start(out=woutZ, in_=d["woutZ"])

        # ---- state
        hist = []   # h~ ring, 4 slots of BC
        ct = []     # c~ = 2c
        acc = []    # accum_out scratch for the custom DVE op
        for ch in range(NCHUNK):
            hh = state.tile([H, 4 * BC], BF16, tag=f"hist{ch}")
            c = state.tile([H, BC], BF16, tag=f"ct{ch}")
            a = state.tile([H, 1], F32, tag=f"acc{ch}")
            nc.vector.memset(hh, 0.0)
            nc.vector.memset(c, 0.0)
            hist.append(hh)
            ct.append(c)
            acc.append(a)

        xqt = [[None] * NBLK, [None] * NBLK]   # xq block tiles per chunk

        sl = [None, None]     # ACT1 output (tanh of all gates)
        pps = [None, None]

        def front(s, ch):
            """Input MM + 4 whh MMs + one tanh ACT over all gates."""
            phase_p = s < NP
            if phase_p and s % BLK == 0:
                blk = s // BLK
                if blk == 0:
                    xqt[ch][0] = xpool.tile([8, BLK * 4 * BC], BF16,
                                            tag=f"xqt{ch}", name=f"xqt{ch}_0")
                    nc.sync.dma_start(out=xqt[ch][0], in_=d["xq"][0, ch])
                if blk + 1 < NBLK:
                    xqt[ch][blk + 1] = xpool.tile([8, BLK * 4 * BC], BF16,
                                                  tag=f"xqt{ch}",
                                                  name=f"xqt{ch}_{blk + 1}")
                    nc.sync.dma_start(out=xqt[ch][blk + 1],
                                      in_=d["xq"][blk + 1, ch])

            gates = gpool.tile([H, 4 * H], F32, tag=f"g{ch}", name=f"g{ch}_{s}")
            whh = whhT_p if phase_p else whhT_h
            lp2 = lp2_p if phase_p else lp2_h
            if phase_p:
                j = s % BLK
                rhs = xqt[ch][s // BLK][:, j * 4 * BC:(j + 1) * 4 * BC]
            else:
                rhs = hq
            nc.tensor.matmul(gates, lp2, rhs, start=True, stop=True,
                             skip_group_check=True)
            hprev = hist[ch][:, ((s - 1) % 4) * BC:((s - 1) % 4 + 1) * BC]
            for k in range(4):
                nc.tensor.matmul(gates[:, k * H:(k + 1) * H],
                                 whh[:, k * H:(k + 1) * H], hprev,
                                 start=False, stop=(k == 3),
                                 skip_group_check=True)
            S = work.tile([H, 4 * H], BF16, tag=f"S{ch}", name=f"S{ch}_{s}")
            nc.scalar.activation(S, gates, AF.Tanh)
            sl[ch] = S

        def back(s, ch):
            """Cell update + h~ + batched predictions."""
            S = sl[ch]
            Si = S[:, 0:H]
            Sf = S[:, H:2 * H]
            Tg = S[:, 2 * H:3 * H]
            So = S[:, 3 * H:4 * H]

            u = work.tile([H, BC], BF16, tag=f"u{ch}", name=f"u{ch}_{s}")
            nc.vector.scalar_tensor_tensor(u, Si, 1.0, Tg, ALU.add, ALU.mult)
            v = work.tile([H, BC], BF16, tag=f"v{ch}", name=f"v{ch}_{s}")
            nc.vector.affine_mul_reduce(v, acc[ch], Sf, ct[ch], 0.5, 0.5)
            nc.vector.tensor_add(ct[ch], u, v)
            tcn = work.tile([H, BC], BF16, tag=f"tcn{ch}", name=f"tcn{ch}_{s}")
            nc.scalar.activation(tcn, ct[ch], AF.Tanh, scale=0.5)
            hslot = hist[ch][:, (s % 4) * BC:(s % 4 + 1) * BC]
            nc.vector.scalar_tensor_tensor(hslot, So, 1.0, tcn,
                                           ALU.add, ALU.mult)

            # Predictions: every 4 steps one matmul over the h~ ring; row
            # placement in PSUM via the shifted zero-padded W_out stationary.
            if s % 4 == 3 or s == NT - 1:
                G = s // 4
                r = G % 32
                n = (s % 4 + 1) * BC
                if r == 0:
                    pps[ch] = ppool.tile([H, 4 * BC], F32, tag=f"pps{ch}",
                                         name=f"pps{ch}_{s}")
                nc.tensor.matmul(pps[ch][:, 0:n],
                                 woutZ[:, H - r:2 * H - r],
                                 hist[ch][:, 0:n],
                                 start=(r == 0), stop=(r == 31 or s == NT - 1),
                                 skip_group_check=True)
                if r == 31 or s == NT - 1:
                    e = G // 32
                    pc = work.tile([32, 4 * BC], F32, tag=f"pc{ch}",
                                   name=f"pc{ch}_{s}")
                    nc.vector.tensor_copy(pc, pps[ch][0:32, :])
                    nc.sync.dma_start(out=d["preds"][e, ch], in_=pc)

        for s in range(NT):
            front(s, 0)
            if s > 0:
                back(s - 1, 1)
            front(s, 1)
            back(s, 0)
        back(NT - 1, 1)


@functools.lru_cache(maxsize=2)
def _program(NP, NH):
    nc = bacc.Bacc("TRN2", target_bir_lowering=False, debug=False,
                   num_devices=NCORES)
    NT = NP + NH
    NEP = (NT + 127) // 128
    NBLK = (NP + BLK - 1) // BLK
    d = {
        "whhT_p": nc.dram_tensor("whhT_p", [H, 4 * H], BF16,
                                 kind="ExternalInput").ap(),
        "whhT_h": nc.dram_tensor("whhT_h", [H, 4 * H], BF16,
                                 kind="ExternalInput").ap(),
        "lp2_p": nc.dram_tensor("lp2_p", [8, H], BF16,
                                kind="ExternalInput").ap(),
        "lp2_h": nc.dram_tensor("lp2_h", [8, H], BF16,
                                kind="ExternalInput").ap(),
        "hq": nc.dram_tensor("hq", [8, 4 * BC], BF16,
                             kind="ExternalInput").ap(),
        "woutZ": nc.dram_tensor("woutZ", [H, 2 * H], BF16,
                                kind="ExternalInput").ap(),
        "xq": nc.dram_tensor("xq", [NBLK, 2, 8, BLK * 4 * BC], BF16,
                             kind="ExternalInput").ap(),
        "preds": nc.dram_tensor("preds", [NEP, NCHUNK, 32, 4 * BC], F32,
                                kind="ExternalOutput").ap(),
    }
    with tile.TileContext(nc) as tc:
        _build_body(tc, d, NP, NH)
    nc.compile()
    return nc, d


def _host_prep(y_flow, W_ih, W_hh, b_ih, b_hh, W_out, b_out, NP):
    """Build per-core input maps with the all-tanh / doubled-state scaling."""
    bf = ml_dtypes.bfloat16
    W_ih = np.asarray(W_ih, np.float64)
    W_hh = np.asarray(W_hh, np.float64)
    W_out = np.asarray(W_out, np.float64)
    bias = np.asarray(b_ih, np.float64) + np.asarray(b_hh, np.float64)
    b_out = np.asarray(b_out, np.float64)

    W_hh_H = W_hh + W_ih @ W_out              # [4H, H] fold p_{t-1} feedback
    bias_H = bias + W_ih[:, 0] * b_out[0]

    # gate row scaling: i,f,o rows * 0.5 (tanh(x/2) trick); g rows * 1
    gs = np.ones(4 * H)
    gs[0:H] = 0.5          # i
    gs[H:2 * H] = 0.5      # f
    gs[3 * H:4 * H] = 0.5  # o
    # h~ = 2h absorption: scale W_hh columns (h input dim) by 0.5
    whhT_p = (W_hh * gs[:, None] * 0.5).T     # [H, 4H]
    whhT_h = (W_hh_H * gs[:, None] * 0.5).T

    # K=8 input stationary: rows 2g = W_ih slice, rows 2g+1 = bias slice
    lp2_p = np.zeros((8, H))
    lp2_h = np.zeros((8, H))
    for g in range(4):
        lp2_p[2 * g] = (W_ih[:, 0] * gs)[g * H:(g + 1) * H]
        lp2_p[2 * g + 1] = (bias * gs)[g * H:(g + 1) * H]
        lp2_h[2 * g + 1] = (bias_H * gs)[g * H:(g + 1) * H]

    # constant phase-H rhs: ones in rows 2g+1, block g
    hqm = np.zeros((8, 4 * BC))
    for g in range(4):
        hqm[2 * g + 1, g * BC:(g + 1) * BC] = 1.0

    # predictions: p = W_out h = (0.5 W_out) h~
    woutZ = np.zeros((H, 2 * H))
    woutZ[:, H] = W_out[0] * 0.5

    NBLK = (NP + BLK - 1) // BLK
    y = np.asarray(y_flow, np.float32)[:, :, 0]   # [B, T]
    in_maps = []
    const_map = {
        "whhT_p": whhT_p.astype(bf), "whhT_h": whhT_h.astype(bf),
        "lp2_p": lp2_p.astype(bf), "lp2_h": lp2_h.astype(bf),
        "hq": hqm.astype(bf), "woutZ": woutZ.astype(bf),
    }
    for core in range(NCORES):
        yc = y[core * BS:(core + 1) * BS]         # [BS, T]
        xq = np.zeros((NBLK, NCHUNK, 8, BLK * 4 * BC), np.float32)
        for ch in range(NCHUNK):
            yb = np.zeros((NBLK * BLK, BC), np.float32)
            yb[:NP] = yc[ch * BC:(ch + 1) * BC, :NP].T
            yb = yb.reshape(NBLK, BLK, BC)
            ones = np.zeros((NBLK * BLK, BC), np.float32)
            ones[:NP] = 1.0
            ones = ones.reshape(NBLK, BLK, BC)
            for g in range(4):
                # rows 2g: y values in block g; rows 2g+1: ones
                blkv = xq[:, ch, 2 * g].reshape(NBLK, BLK, 4 * BC)
                blkv[:, :, g * BC:(g + 1) * BC] = yb
                blko = xq[:, ch, 2 * g + 1].reshape(NBLK, BLK, 4 * BC)
                blko[:, :, g * BC:(g + 1) * BC] = ones
        in_maps.append(dict(const_map, xq=xq.astype(bf)))
    return in_maps


def kernel(y_flow, x_dyn, W_ih, W_hh, b_ih, b_hh, W_out, b_out, twin_idx,
           _trace=False):
    twin = int(twin_idx)
    assert twin == 256, f"kernel hardcodes twin_idx=256, got {twin}"
    B, T, _ = y_flow.shape
    assert (B, T) == (2048, 512)
    NP, NH = twin - 1, T - twin
    NT = NP + NH

    nc, _ = _program(NP, NH)
    in_maps = _host_prep(y_flow, W_ih, W_hh, b_ih, b_hh, W_out, b_out, NP)
    res = run_bass_kernel_spmd(nc, in_maps, core_ids=list(range(NCORES)),
                               trace=_trace)

    b_out = np.asarray(b_out, np.float32)
    out = np.empty((B, NT, 1), np.float32)
    for core in range(NCORES):
        p = np.asarray(res.results[core]["preds"], np.float32)
        nep = p.shape[0]
        a = p.reshape(nep, NCHUNK, 32, 4, BC)      # [e, ch, r, j, b]
        for ch in range(NCHUNK):
            blk = a[:, ch].transpose(3, 0, 1, 2).reshape(BC, -1)[:, :NT]
            out[core * BS + ch * BC: core * BS + (ch + 1) * BC, :, 0] = \
                blk + b_out[0]
    if _trace:
        kernel._last_results = res
    return out
